# revision 34
# baseline (speedup 1.0000x reference)
"""MoE gate routing kernel for Trainium2 (Bass/Tile), 8-way token-sharded.

Computes, for x = hidden_states.reshape(-1, H) and gate weight W [E, H]:
    logits = x @ W.T            # [T, E]
    top-8 of softmax(logits) with renormalized weights
Returns (topk_weight [T, 8] f32, topk_idx [T, 8] i32), matching the reference.

Math note: softmax then top-k + renormalize equals top-k on logits followed
by softmax over just those 8 logits (the global partition function cancels;
the reference's +1e-20 is negligible since the max prob >= 1/64).

Precision: x and W are split on the host into bf16 hi + bf16 lo halves
(x ~= xh + xl to ~2^-18 relative). logits are computed as three accumulating
bf16 matmul chains xh@wh + xh@wl + xl@wh in fp32 PSUM; the dropped xl@wl
term is ~2^-18 relative, giving near-fp32 logits (top-8 flips only on
near-exact ties; simulated rel_i ~4e-3, well under the 2e-2 gate). bf16
matmuls stream 1 column/cycle on the PE where plain fp32 needs 4.

Layout: the host transposes x to xT [H, T] and ships bf16 halves, so the
kernel needs NO on-device transposes of x (the dominant PE cost of an
fp32 version). DMA traffic is unchanged vs fp32 x (2 halves x 2 bytes).

Per-core schedule (2048 tokens = 4 slabs x 512 tokens):
  - wTh/wTl staged in SBUF once ([128, 32*64] chunk-major).
  - Per slab: 8 input DMAs (4 sub-blocks x {xh, xl}), each [128, 8x512]
    bf16 with fully contiguous 8KB per-partition lines (slab-major host
    layout); 96 accumulating matmuls (3 chains x 32 k-chunks) col-paired
    via tile_position into PSUM [128, 512] (each half of the PE array
    carries 1.5 chains; partials land in either PSUM half arbitrarily).
  - Epilogue, software-pipelined one slab behind the mains so the PE
    stream never stalls on it: DVE copy of the PSUM halves to SBUF, an
    exact fp32 merge matmul (stacked identities) adds the halves, 4 exact
    fp32 PE transposes back to [tokens, experts], then per 128-token
    quarter: DVE max8 + max-index (reading PSUM directly), ACT exp with
    accumulate, DVE reciprocal + scale. Outputs go out on the ACT hwdge
    so they never head-of-line block the SP input-prefetch stream.
  - PSUM: 4 lgt buffers decouple slab k's mains from slab k-2's epilogue.
"""

import numpy as np

import concourse.bass as bass
import concourse.mybir as mybir
from concourse import masks
from concourse.bass_utils import run_bass_kernel_spmd
from concourse.tile import TileContext

P = 128          # SBUF partitions
H = 4096         # hidden dim
E = 64           # experts
K = 8            # top-k
N_CORES = 8
T_TOTAL = 4 * 4096
T_CORE = T_TOTAL // N_CORES   # 2048
SLAB = 4 * P                  # 512 tokens per slab
N_SLAB = T_CORE // SLAB       # 4
N_KC = H // P                 # 32 contraction chunks
N_SUB = 4                     # DMA sub-blocks per slab
KC_SUB = N_KC // N_SUB        # 8 chunks per sub-block

F32 = mybir.dt.float32
BF16 = mybir.dt.bfloat16
U32 = mybir.dt.uint32
EXP = mybir.ActivationFunctionType.Exp


def build_bass(loop_reps=None, xin_bufs=None, lgt_bufs=4, pair=True, n_sub=N_SUB,
               xl_on_act=False, sm_bufs=3, stack=False, coalesce=True):
    kc_sub = N_KC // n_sub
    if xin_bufs is None:
        xin_bufs = 2 * n_sub
    nc = bass.Bass()
    # x^T bf16 halves, slab-major so each DMA line is 8KB contiguous:
    # element [s, p, kc, t] = xT[kc*128 + p, s*SLAB + t]
    xh = nc.declare_dram_parameter("xh", [N_SLAB, P, N_KC, SLAB], BF16, isOutput=False)
    xl = nc.declare_dram_parameter("xl", [N_SLAB, P, N_KC, SLAB], BF16, isOutput=False)
    # W^T bf16 halves, chunk-major [kc, p, e]; whl = host-interleaved
    # [wh_kc | wl_kc] stacked along the expert axis
    wh = nc.declare_dram_parameter("wh", [N_KC, P, E], BF16, isOutput=False)
    wl = nc.declare_dram_parameter("wl", [N_KC, P, E], BF16, isOutput=False)
    whl = nc.declare_dram_parameter("whl", [N_KC, P, 2 * E], BF16, isOutput=False)
    out_w = nc.declare_dram_parameter("out_w", [T_CORE, K], F32, isOutput=True)
    out_i = nc.declare_dram_parameter("out_i", [T_CORE, K], U32, isOutput=True)

    with TileContext(nc) as tc:
        with (
            tc.tile_pool(name="singles", bufs=1) as singles,
            tc.tile_pool(name="xin", bufs=xin_bufs) as x_pool,
            tc.tile_pool(name="lgtp", bufs=lgt_bufs, space="PSUM") as lgt_psum,
            tc.tile_pool(name="mgp", bufs=1, space="PSUM") as mg_psum,
            tc.tile_pool(name="lgqp", bufs=2, space="PSUM") as lgq_psum,
            tc.tile_pool(name="sm", bufs=sm_bufs) as sm_pool,
        ):
            identity = singles.tile([P, P], F32)
            masks.make_identity(nc, identity[:])
            # merge operand: [I64; I64] stacked -> adds PSUM halves
            mergeM = singles.tile([P, E], F32)
            nc.gpsimd.memset(mergeM[:], 0.0)
            masks.make_identity(nc, mergeM[0:E, 0:E], nomemset=True)
            masks.make_identity(nc, mergeM[E : 2 * E, 0:E], nomemset=True)

            if stack:
                # host-interleaved stationary [wh_kc | wl_kc] per chunk: one
                # full-width [128, 128] weight computes x@wh (psum 0:64) and
                # x@wl (psum 64:128) from a single moving stream
                whl_s = singles.tile([P, N_KC * 2 * E], BF16)
                nc.sync.dma_start(
                    out=whl_s[:].rearrange("p (kc e) -> p kc e", kc=N_KC),
                    in_=whl[:, :, :].transpose([1, 0, 2]),
                )
            else:
                whs = singles.tile([P, N_KC * E], BF16)
                wls = singles.tile([P, N_KC * E], BF16)
                nc.sync.dma_start(
                    out=whs[:].rearrange("p (kc e) -> p kc e", kc=N_KC),
                    in_=wh[:, :, :].transpose([1, 0, 2]),
                )
                nc.sync.dma_start(
                    out=wls[:].rearrange("p (kc e) -> p kc e", kc=N_KC),
                    in_=wl[:, :, :].transpose([1, 0, 2]),
                )

            def emit_dma(s):
                xh_t, xl_t = [], []
                for b in range(n_sub):
                    xht = x_pool.tile([P, kc_sub * SLAB], BF16, tag="xh")
                    nc.sync.dma_start(
                        out=xht[:],
                        in_=xh[s, :, b * kc_sub : (b + 1) * kc_sub, :].rearrange(
                            "p kc t -> p (kc t)"
                        ),
                    )
                    xh_t.append(xht)
                    xlt = x_pool.tile([P, kc_sub * SLAB], BF16, tag="xl")
                    xl_eng = nc.scalar if xl_on_act else nc.sync
                    xl_eng.dma_start(
                        out=xlt[:],
                        in_=xl[s, :, b * kc_sub : (b + 1) * kc_sub, :].rearrange(
                            "p kc t -> p (kc t)"
                        ),
                    )
                    xl_t.append(xlt)
                return xh_t, xl_t

            def emit_mains(xh_t, xl_t):
                if stack:
                    # One [128,128] stationary [wh|wl] per chunk; two matmuls
                    # (moving xh, then xl) accumulate all four products
                    # xh@wh + xl@wh (psum 0:64) and xh@wl + xl@wl (64:128).
                    lgt = lgt_psum.tile([P, SLAB], F32)
                    n_mm = 2 * N_KC
                    i_mm = 0
                    for kc in range(N_KC):
                        b, j = divmod(kc, kc_sub)
                        w_st = whl_s[:, kc * 2 * E : (kc + 1) * 2 * E]
                        for x_mv in (xh_t[b][:, j * SLAB : (j + 1) * SLAB],
                                     xl_t[b][:, j * SLAB : (j + 1) * SLAB]):
                            nc.tensor.matmul(
                                lgt[:], w_st, x_mv,
                                start=(i_mm == 0), stop=(i_mm == n_mm - 1),
                            )
                            i_mm += 1
                    return lgt
                # Three bf16 chains (xh@wh + xl@wh + xh@wl) col-paired on the
                # 128-wide PE array: M=64 uses only half the columns, so two
                # matmuls run concurrently via tile_position (0,0)/(0,64).
                # Alternate which half carries 2-of-3 per k-chunk so both
                # halves do 1.5 matmuls/chunk; partials split arbitrarily
                # between PSUM halves and are summed in the merge stage.
                first = {0: True, 1: True}
                seq = []
                for kc in range(N_KC):
                    b, j = divmod(kc, kc_sub)
                    xh_mov = xh_t[b][:, j * SLAB : (j + 1) * SLAB]
                    xl_mov = xl_t[b][:, j * SLAB : (j + 1) * SLAB]
                    w_hi = whs[:, kc * E : (kc + 1) * E]
                    w_lo = wls[:, kc * E : (kc + 1) * E]
                    if not pair:
                        seq += [(0, w_hi, xh_mov), (0, w_hi, xl_mov),
                                (0, w_lo, xh_mov)]
                    elif kc % 2 == 0:
                        seq += [(0, w_hi, xh_mov), (1, w_lo, xh_mov),
                                (0, w_hi, xl_mov)]
                    else:
                        seq += [(1, w_hi, xh_mov), (0, w_lo, xh_mov),
                                (1, w_hi, xl_mov)]
                lgt = lgt_psum.tile([P if pair else E, SLAB], F32)
                last_of = {h: max((i for i, m in enumerate(seq) if m[0] == h),
                                  default=None)
                           for h in (0, 1)}
                for i, (half, w_st, x_mv) in enumerate(seq):
                    # per-half start/stop (each clears/ends its own partition
                    # range's has_written); CoreSim's group check keys zero
                    # regions without base partition, so skip it.
                    nc.tensor.matmul(
                        lgt[half * E : (half + 1) * E, :], w_st, x_mv,
                        start=first[half], stop=(i == last_of[half]),
                        tile_position=(0, half * E),
                        skip_group_check=True,
                    )
                    first[half] = False
                return lgt

            def emit_epi(s, lgt):
                if pair or stack:
                    # merge halves exactly (fp32 matmul, stacked identities),
                    # then exact fp32 transposes back to [tokens, experts]
                    lgt_sb = sm_pool.tile([P, SLAB], F32, tag="lgt_sb")
                    nc.vector.tensor_copy(lgt_sb[:], lgt[:])
                    mg = mg_psum.tile([E, SLAB], F32, tag="epi")
                    nc.tensor.matmul(
                        mg[:], mergeM[:], lgt_sb[:], start=True, stop=True,
                        tile_position=(0, 0),
                    )
                    mg_sb = sm_pool.tile([E, SLAB], F32, tag="mg_sb")
                    nc.vector.tensor_copy(mg_sb[:], mg[:])
                else:
                    mg_sb = sm_pool.tile([E, SLAB], F32, tag="mg_sb")
                    nc.vector.tensor_copy(mg_sb[:], lgt[:])
                lgq = lgq_psum.tile([P, 4 * E], F32)
                for q in range(4):
                    nc.tensor.matmul(
                        lgq[:, q * E : (q + 1) * E],
                        mg_sb[:, q * P : (q + 1) * P],
                        identity[:E, :E],
                        is_transpose=True,
                        start=(q == 0),
                        stop=(q == 3),
                    )

                # top-8 in phases across the 4 quarters so the in-order DVE
                # stream never head-of-line blocks on the ACT exp
                t8v, t8i, nmax, e8, s1 = [], [], [], [], []
                for q in range(4):
                    lg = lgq[:, q * E : (q + 1) * E]
                    v = sm_pool.tile([P, K], F32, tag="t8v")
                    nc.vector.max(out=v[:], in_=lg)
                    i_ = sm_pool.tile([P, K], U32, tag="t8i")
                    nc.vector.max_index(out=i_[:], in_max=v[:], in_values=lg)
                    m = sm_pool.tile([P, 1], F32, tag="nmax")
                    nc.vector.tensor_scalar_mul(m[:], v[:, 0:1], -1.0)
                    t8v.append(v); t8i.append(i_); nmax.append(m)
                for q in range(4):
                    e = sm_pool.tile([P, K], F32, tag="e8")
                    a = sm_pool.tile([P, 1], F32, tag="s1")
                    nc.scalar.activation(
                        e[:], t8v[q][:], EXP, bias=nmax[q][:], scale=1.0,
                        accum_out=a[:],
                    )
                    e8.append(e); s1.append(a)
                for q in range(4):
                    r1 = sm_pool.tile([P, 1], F32, tag="r1")
                    nc.vector.reciprocal(r1[:], s1[q][:])
                    w8 = sm_pool.tile([P, K], F32, tag="w8")
                    nc.vector.tensor_scalar_mul(w8[:], e8[q][:], r1[:])
                    tq = s * SLAB + q * P
                    # outputs go out on the ACT hwdge so they never block the
                    # SP stream's input prefetch
                    nc.scalar.dma_start(out=out_w[tq : tq + P, :], in_=w8[:])
                    nc.scalar.dma_start(out=out_i[tq : tq + P, :], in_=t8i[q][:])

            def main_body():
                # software pipeline: DMA(s+1) and mains(s) are emitted before
                # epilogue(s-1) so the PE stream never stalls on the epilogue's
                # cross-engine dependency chain
                tiles = {0: emit_dma(0)}
                lgts = {}
                for s in range(N_SLAB):
                    if s + 1 < N_SLAB:
                        tiles[s + 1] = emit_dma(s + 1)
                    lgts[s] = emit_mains(*tiles.pop(s))
                    if s >= 1:
                        emit_epi(s - 1, lgts.pop(s - 1))
                emit_epi(N_SLAB - 1, lgts.pop(N_SLAB - 1))

            if loop_reps is None:
                main_body()
            else:
                with tc.For_i(0, loop_reps, 1):
                    main_body()

    if coalesce:
        _coalesce_sem_incs(nc)
    _legalize_waits(nc)
    return nc


def _coalesce_sem_incs(nc):
    """Coalesce per-instruction semaphore increments.

    Tile attaches a +1 sem-inc to every PE/DVE/ACT instruction for buffer
    rotation tracking; serialized EVT_SEM register writes cost ~26ns each
    (404 matmul incs ~= 10us here). Engines complete instructions in order
    and waits are immediate thresholds, so a semaphore only needs its
    increments at instructions whose cumulative count crosses some waited
    threshold. Walrus requires UpdateValue == 1, so instead of bulk incs we
    keep one +1 at each threshold-crossing instruction and renumber every
    wait threshold to count kept increments: the semaphore reaches each
    (renumbered) threshold at exactly the same instruction completion as
    before. Only applied to semaphores whose increments all come from one
    engine's non-DMA instructions in one block, are all sem-inc(+1)
    immediates, and whose waits are all sem-ge-imm (resets, decs or
    register operands disqualify the semaphore)."""
    waits = {}
    incs = {}
    bad = set()
    for f in nc.m.functions:
        for bi, blk in enumerate(f.blocks):
            for inst in blk.instructions:
                si = getattr(inst, "sync_info", None)
                if si is None:
                    continue
                for w in si.on_wait or []:
                    if w.sync_type != "semaphore":
                        continue
                    if w.wait_mode != "sem-ge-imm" or w.wait_value is None:
                        bad.add(w.id)
                    else:
                        waits.setdefault(w.id, []).append(w)
                is_dma = "DMA" in type(inst).__name__.upper()
                for u in si.on_update or []:
                    if u.sync_type != "semaphore":
                        continue
                    if (
                        u.update_mode != "sem-inc"
                        or u.update_reg is not None
                        or u.update_value != 1
                        or is_dma
                    ):
                        bad.add(u.id)
                    else:
                        incs.setdefault(u.id, []).append((bi, inst.engine, inst, u))
    import bisect

    for sid, lst in incs.items():
        if sid in bad:
            continue
        if len({bi for bi, _, _, _ in lst}) != 1:
            continue
        if len({str(e) for _, e, _, _ in lst}) != 1:
            continue
        if len(lst) < 8:
            continue
        wlist = waits.get(sid, [])
        thresholds = sorted({w.wait_value for w in wlist})
        if thresholds and thresholds[-1] > len(lst):
            continue  # wait beyond total: leave untouched
        kept_cums = []
        ti = 0
        for idx in range(len(lst)):
            cum = idx + 1
            if ti < len(thresholds) and thresholds[ti] <= cum:
                kept_cums.append(cum)
                while ti < len(thresholds) and thresholds[ti] <= cum:
                    ti += 1
        keep_idx = {c - 1 for c in kept_cums}
        for idx, (_, _, inst, u) in enumerate(lst):
            if idx not in keep_idx:
                inst.sync_info.on_update = [
                    x for x in inst.sync_info.on_update if x is not u
                ]
        for w in wlist:
            w.wait_value = bisect.bisect_left(kept_cums, w.wait_value) + 1


def _legalize_waits(nc):
    """Walrus allows only one sem wait on most instruction structs (matmul
    weight-load, DVE/ACT compute, pseudo-DMA, drain). Tile sometimes emits
    more. Fix: hoist excess waits onto standalone EventSemaphore instructions
    inserted just before the owner in its engine stream (same engine ->
    in-order issue preserves semantics)."""
    n = 0
    for f in nc.m.functions:
        for blk in f.blocks:
            out = []
            changed = False
            for i in blk.instructions:
                si = getattr(i, "sync_info", None)
                ow = list(si.on_wait) if (si is not None and si.on_wait) else []
                if len(ow) > 1:
                    while len(ow) > 1:
                        w = ow.pop(0)
                        out.append(
                            mybir.InstEventSemaphore(
                                name=f"I-whoist-{n}",
                                engine=i.engine,
                                ins=[],
                                outs=[],
                                sync_info=mybir.SyncInfo(on_wait=[w], on_update=[]),
                            )
                        )
                        n += 1
                    si.on_wait = ow
                    changed = True
                out.append(i)
            if changed:
                blk.instructions = out
    return nc


def _bf16_split(a_f32):
    """Split fp32 array into (hi, lo) bf16 halves, RNE, as bf16 views."""
    import ml_dtypes

    def rne_bf16(f):
        bits = f.view(np.uint32)
        lsb = (bits >> np.uint32(16)) & np.uint32(1)
        rnd = bits + np.uint32(0x7FFF) + lsb
        return (rnd >> np.uint32(16)).astype(np.uint16)

    hi_u16 = rne_bf16(a_f32)
    hi_f32 = (hi_u16.astype(np.uint32) << np.uint32(16)).view(np.float32)
    lo_u16 = rne_bf16(a_f32 - hi_f32)
    return hi_u16.view(ml_dtypes.bfloat16), lo_u16.view(ml_dtypes.bfloat16)


_NC = None


def _get_nc():
    global _NC
    if _NC is None:
        _NC = build_bass()
    return _NC


def host_prepare(hidden_states, weight):
    """Shard + transpose + bf16-split the full inputs into per-core maps."""
    hs = np.asarray(hidden_states, dtype=np.float32).reshape(T_TOTAL, H)
    wt = np.ascontiguousarray(np.asarray(weight, dtype=np.float32).T)  # [H, E]
    wh_u, wl_u = _bf16_split(wt)
    wh3 = np.ascontiguousarray(wh_u.reshape(N_KC, P, E))
    wl3 = np.ascontiguousarray(wl_u.reshape(N_KC, P, E))
    whl3 = np.ascontiguousarray(np.concatenate([wh3, wl3], axis=2))

    in_maps = []
    for c in range(N_CORES):
        xc = np.ascontiguousarray(hs[c * T_CORE : (c + 1) * T_CORE, :].T)  # [H, Tc]
        xh_u, xl_u = _bf16_split(xc)

        def slab_major(a):
            a4 = a.reshape(N_KC, P, N_SLAB, SLAB).transpose(2, 1, 0, 3)
            return np.ascontiguousarray(a4)

        in_maps.append(
            {
                "xh": slab_major(xh_u),
                "xl": slab_major(xl_u),
                "wh": wh3,
                "wl": wl3,
                "whl": whl3,
            }
        )
    return in_maps


def kernel(hidden_states, weight, **run_kwargs):
    in_maps = host_prepare(hidden_states, weight)
    nc = _get_nc()
    res = run_bass_kernel_spmd(nc, in_maps, core_ids=list(range(N_CORES)), **run_kwargs)
    topk_weight = np.concatenate([r["out_w"] for r in res.results], axis=0)
    topk_idx = np.concatenate(
        [r["out_i"].astype(np.int32) for r in res.results], axis=0
    )
    if run_kwargs:
        kernel.last_result = res
    return topk_weight, topk_idx


# revision 36
# speedup vs baseline: 1.6036x; 1.6036x over previous
"""MoE gate routing kernel for Trainium2 (Bass/Tile), 8-way token-sharded.

Computes, for x = hidden_states.reshape(-1, H) and gate weight W [E, H]:
    logits = x @ W.T            # [T, E]
    top-8 of softmax(logits) with renormalized weights
Returns (topk_weight [T, 8] f32, topk_idx [T, 8] i32), matching the reference.

Math note: softmax then top-k + renormalize equals top-k on logits followed
by softmax over just those 8 logits (the global partition function cancels;
the reference's +1e-20 is negligible since the max prob >= 1/64).

Precision: x and W are split on the host into bf16 hi + bf16 lo halves
(x ~= xh + xl to ~2^-18 relative). logits are computed as three accumulating
bf16 matmul chains xh@wh + xh@wl + xl@wh in fp32 PSUM; the dropped xl@wl
term is ~2^-18 relative, giving near-fp32 logits (top-8 flips only on
near-exact ties; simulated rel_i ~4e-3, well under the 2e-2 gate). bf16
matmuls stream 1 column/cycle on the PE where plain fp32 needs 4.

Layout: the host transposes x to xT [H, T] and ships bf16 halves, so the
kernel needs NO on-device transposes of x (the dominant PE cost of an
fp32 version). DMA traffic is unchanged vs fp32 x (2 halves x 2 bytes).

Per-core schedule (2048 tokens = 4 slabs x 512 tokens):
  - wTh/wTl staged in SBUF once ([128, 32*64] chunk-major).
  - Per slab: 4 input DMAs (one fused {xh, xl} pair per sub-block), each
    [128, 2x8x512] bf16 with contiguous 8KB per-partition lines
    (slab-major host layout; fusing halved the DMA instruction count and
    measured 12% faster); 96 accumulating matmuls col-paired
    via tile_position into PSUM [128, 512] (each half of the PE array
    carries 1.5 chains; partials land in either PSUM half arbitrarily).
  - Epilogue, software-pipelined one slab behind the mains so the PE
    stream never stalls on it: DVE copy of the PSUM halves to SBUF, an
    exact fp32 merge matmul (stacked identities) adds the halves, 4 exact
    fp32 PE transposes back to [tokens, experts], then per 128-token
    quarter: DVE max8 + max-index (reading PSUM directly), ACT exp with
    accumulate, DVE reciprocal + scale. Outputs go out on the ACT hwdge
    so they never head-of-line block the SP input-prefetch stream.
  - PSUM: 4 lgt buffers decouple slab k's mains from slab k-2's epilogue.
"""

import numpy as np

import concourse.bass as bass
import concourse.mybir as mybir
from concourse import masks
from concourse.bass_utils import run_bass_kernel_spmd
from concourse.tile import TileContext

P = 128          # SBUF partitions
H = 4096         # hidden dim
E = 64           # experts
K = 8            # top-k
N_CORES = 8
T_TOTAL = 4 * 4096
T_CORE = T_TOTAL // N_CORES   # 2048
SLAB = 4 * P                  # 512 tokens per slab
N_SLAB = T_CORE // SLAB       # 4
N_KC = H // P                 # 32 contraction chunks
N_SUB = 4                     # DMA sub-blocks per slab
KC_SUB = N_KC // N_SUB        # 8 chunks per sub-block

F32 = mybir.dt.float32
BF16 = mybir.dt.bfloat16
U32 = mybir.dt.uint32
EXP = mybir.ActivationFunctionType.Exp


def build_bass(loop_reps=None, xin_bufs=None, lgt_bufs=4, pair=True, n_sub=N_SUB,
               xl_on_act=False, sm_bufs=3, stack=False, coalesce=True,
               fuse_x=True):
    kc_sub = N_KC // n_sub
    if xin_bufs is None:
        xin_bufs = 2 * n_sub
    nc = bass.Bass()
    # x^T bf16 halves, slab-major so each DMA line is 8KB contiguous:
    # element [s, p, kc, t] = xT[kc*128 + p, s*SLAB + t]
    xh = nc.declare_dram_parameter("xh", [N_SLAB, P, N_KC, SLAB], BF16, isOutput=False)
    xl = nc.declare_dram_parameter("xl", [N_SLAB, P, N_KC, SLAB], BF16, isOutput=False)
    # fused variant: both halves in one tensor -> one DMA per sub-block
    x2 = nc.declare_dram_parameter(
        "x2", [N_SLAB, P, 2, N_KC, SLAB], BF16, isOutput=False
    )
    # W^T bf16 halves, chunk-major [kc, p, e]; whl = host-interleaved
    # [wh_kc | wl_kc] stacked along the expert axis
    wh = nc.declare_dram_parameter("wh", [N_KC, P, E], BF16, isOutput=False)
    wl = nc.declare_dram_parameter("wl", [N_KC, P, E], BF16, isOutput=False)
    whl = nc.declare_dram_parameter("whl", [N_KC, P, 2 * E], BF16, isOutput=False)
    out_w = nc.declare_dram_parameter("out_w", [T_CORE, K], F32, isOutput=True)
    out_i = nc.declare_dram_parameter("out_i", [T_CORE, K], U32, isOutput=True)

    with TileContext(nc) as tc:
        with (
            tc.tile_pool(name="singles", bufs=1) as singles,
            tc.tile_pool(name="xin", bufs=xin_bufs) as x_pool,
            tc.tile_pool(name="lgtp", bufs=lgt_bufs, space="PSUM") as lgt_psum,
            tc.tile_pool(name="mgp", bufs=1, space="PSUM") as mg_psum,
            tc.tile_pool(name="lgqp", bufs=2, space="PSUM") as lgq_psum,
            tc.tile_pool(name="sm", bufs=sm_bufs) as sm_pool,
        ):
            identity = singles.tile([P, P], F32)
            masks.make_identity(nc, identity[:])
            # merge operand: [I64; I64] stacked -> adds PSUM halves
            mergeM = singles.tile([P, E], F32)
            nc.gpsimd.memset(mergeM[:], 0.0)
            masks.make_identity(nc, mergeM[0:E, 0:E], nomemset=True)
            masks.make_identity(nc, mergeM[E : 2 * E, 0:E], nomemset=True)

            if stack:
                # host-interleaved stationary [wh_kc | wl_kc] per chunk: one
                # full-width [128, 128] weight computes x@wh (psum 0:64) and
                # x@wl (psum 64:128) from a single moving stream
                whl_s = singles.tile([P, N_KC * 2 * E], BF16)
                nc.sync.dma_start(
                    out=whl_s[:].rearrange("p (kc e) -> p kc e", kc=N_KC),
                    in_=whl[:, :, :].transpose([1, 0, 2]),
                )
            else:
                whs = singles.tile([P, N_KC * E], BF16)
                wls = singles.tile([P, N_KC * E], BF16)
                nc.sync.dma_start(
                    out=whs[:].rearrange("p (kc e) -> p kc e", kc=N_KC),
                    in_=wh[:, :, :].transpose([1, 0, 2]),
                )
                nc.sync.dma_start(
                    out=wls[:].rearrange("p (kc e) -> p kc e", kc=N_KC),
                    in_=wl[:, :, :].transpose([1, 0, 2]),
                )

            def emit_dma(s):
                xh_t, xl_t = [], []
                if fuse_x:
                    for b in range(n_sub):
                        xt = x_pool.tile([P, 2 * kc_sub * SLAB], BF16, tag="x2")
                        nc.sync.dma_start(
                            out=xt[:].rearrange(
                                "p (h kc t) -> p h kc t", h=2, kc=kc_sub
                            ),
                            in_=x2[s, :, :, b * kc_sub : (b + 1) * kc_sub, :],
                        )
                        xh_t.append(xt[:, 0 : kc_sub * SLAB])
                        xl_t.append(xt[:, kc_sub * SLAB : 2 * kc_sub * SLAB])
                    return xh_t, xl_t
                for b in range(n_sub):
                    xht = x_pool.tile([P, kc_sub * SLAB], BF16, tag="xh")
                    nc.sync.dma_start(
                        out=xht[:],
                        in_=xh[s, :, b * kc_sub : (b + 1) * kc_sub, :].rearrange(
                            "p kc t -> p (kc t)"
                        ),
                    )
                    xh_t.append(xht)
                    xlt = x_pool.tile([P, kc_sub * SLAB], BF16, tag="xl")
                    xl_eng = nc.scalar if xl_on_act else nc.sync
                    xl_eng.dma_start(
                        out=xlt[:],
                        in_=xl[s, :, b * kc_sub : (b + 1) * kc_sub, :].rearrange(
                            "p kc t -> p (kc t)"
                        ),
                    )
                    xl_t.append(xlt)
                return xh_t, xl_t

            def emit_mains(xh_t, xl_t):
                if stack:
                    # One [128,128] stationary [wh|wl] per chunk; two matmuls
                    # (moving xh, then xl) accumulate all four products
                    # xh@wh + xl@wh (psum 0:64) and xh@wl + xl@wl (64:128).
                    lgt = lgt_psum.tile([P, SLAB], F32)
                    n_mm = 2 * N_KC
                    i_mm = 0
                    for kc in range(N_KC):
                        b, j = divmod(kc, kc_sub)
                        w_st = whl_s[:, kc * 2 * E : (kc + 1) * 2 * E]
                        for x_mv in (xh_t[b][:, j * SLAB : (j + 1) * SLAB],
                                     xl_t[b][:, j * SLAB : (j + 1) * SLAB]):
                            nc.tensor.matmul(
                                lgt[:], w_st, x_mv,
                                start=(i_mm == 0), stop=(i_mm == n_mm - 1),
                            )
                            i_mm += 1
                    return lgt
                # Three bf16 chains (xh@wh + xl@wh + xh@wl) col-paired on the
                # 128-wide PE array: M=64 uses only half the columns, so two
                # matmuls run concurrently via tile_position (0,0)/(0,64).
                # Alternate which half carries 2-of-3 per k-chunk so both
                # halves do 1.5 matmuls/chunk; partials split arbitrarily
                # between PSUM halves and are summed in the merge stage.
                first = {0: True, 1: True}
                seq = []
                for kc in range(N_KC):
                    b, j = divmod(kc, kc_sub)
                    xh_mov = xh_t[b][:, j * SLAB : (j + 1) * SLAB]
                    xl_mov = xl_t[b][:, j * SLAB : (j + 1) * SLAB]
                    w_hi = whs[:, kc * E : (kc + 1) * E]
                    w_lo = wls[:, kc * E : (kc + 1) * E]
                    if not pair:
                        seq += [(0, w_hi, xh_mov), (0, w_hi, xl_mov),
                                (0, w_lo, xh_mov)]
                    elif kc % 2 == 0:
                        seq += [(0, w_hi, xh_mov), (1, w_lo, xh_mov),
                                (0, w_hi, xl_mov)]
                    else:
                        seq += [(1, w_hi, xh_mov), (0, w_lo, xh_mov),
                                (1, w_hi, xl_mov)]
                lgt = lgt_psum.tile([P if pair else E, SLAB], F32)
                last_of = {h: max((i for i, m in enumerate(seq) if m[0] == h),
                                  default=None)
                           for h in (0, 1)}
                for i, (half, w_st, x_mv) in enumerate(seq):
                    # per-half start/stop (each clears/ends its own partition
                    # range's has_written); CoreSim's group check keys zero
                    # regions without base partition, so skip it.
                    nc.tensor.matmul(
                        lgt[half * E : (half + 1) * E, :], w_st, x_mv,
                        start=first[half], stop=(i == last_of[half]),
                        tile_position=(0, half * E),
                        skip_group_check=True,
                    )
                    first[half] = False
                return lgt

            def emit_epi(s, lgt):
                if pair or stack:
                    # merge halves exactly (fp32 matmul, stacked identities),
                    # then exact fp32 transposes back to [tokens, experts]
                    lgt_sb = sm_pool.tile([P, SLAB], F32, tag="lgt_sb")
                    nc.vector.tensor_copy(lgt_sb[:], lgt[:])
                    mg = mg_psum.tile([E, SLAB], F32, tag="epi")
                    nc.tensor.matmul(
                        mg[:], mergeM[:], lgt_sb[:], start=True, stop=True,
                        tile_position=(0, 0),
                    )
                    mg_sb = sm_pool.tile([E, SLAB], F32, tag="mg_sb")
                    nc.vector.tensor_copy(mg_sb[:], mg[:])
                else:
                    mg_sb = sm_pool.tile([E, SLAB], F32, tag="mg_sb")
                    nc.vector.tensor_copy(mg_sb[:], lgt[:])
                lgq = lgq_psum.tile([P, 4 * E], F32)
                for q in range(4):
                    nc.tensor.matmul(
                        lgq[:, q * E : (q + 1) * E],
                        mg_sb[:, q * P : (q + 1) * P],
                        identity[:E, :E],
                        is_transpose=True,
                        start=(q == 0),
                        stop=(q == 3),
                    )

                # top-8 in phases across the 4 quarters so the in-order DVE
                # stream never head-of-line blocks on the ACT exp
                t8v, t8i, nmax, e8, s1 = [], [], [], [], []
                for q in range(4):
                    lg = lgq[:, q * E : (q + 1) * E]
                    v = sm_pool.tile([P, K], F32, tag="t8v")
                    nc.vector.max(out=v[:], in_=lg)
                    i_ = sm_pool.tile([P, K], U32, tag="t8i")
                    nc.vector.max_index(out=i_[:], in_max=v[:], in_values=lg)
                    m = sm_pool.tile([P, 1], F32, tag="nmax")
                    nc.vector.tensor_scalar_mul(m[:], v[:, 0:1], -1.0)
                    t8v.append(v); t8i.append(i_); nmax.append(m)
                for q in range(4):
                    e = sm_pool.tile([P, K], F32, tag="e8")
                    a = sm_pool.tile([P, 1], F32, tag="s1")
                    nc.scalar.activation(
                        e[:], t8v[q][:], EXP, bias=nmax[q][:], scale=1.0,
                        accum_out=a[:],
                    )
                    e8.append(e); s1.append(a)
                for q in range(4):
                    r1 = sm_pool.tile([P, 1], F32, tag="r1")
                    nc.vector.reciprocal(r1[:], s1[q][:])
                    w8 = sm_pool.tile([P, K], F32, tag="w8")
                    nc.vector.tensor_scalar_mul(w8[:], e8[q][:], r1[:])
                    tq = s * SLAB + q * P
                    # outputs go out on the ACT hwdge so they never block the
                    # SP stream's input prefetch
                    nc.scalar.dma_start(out=out_w[tq : tq + P, :], in_=w8[:])
                    nc.scalar.dma_start(out=out_i[tq : tq + P, :], in_=t8i[q][:])

            def main_body():
                # software pipeline: DMA(s+1) and mains(s) are emitted before
                # epilogue(s-1) so the PE stream never stalls on the epilogue's
                # cross-engine dependency chain
                tiles = {0: emit_dma(0)}
                lgts = {}
                for s in range(N_SLAB):
                    if s + 1 < N_SLAB:
                        tiles[s + 1] = emit_dma(s + 1)
                    lgts[s] = emit_mains(*tiles.pop(s))
                    if s >= 1:
                        emit_epi(s - 1, lgts.pop(s - 1))
                emit_epi(N_SLAB - 1, lgts.pop(N_SLAB - 1))

            if loop_reps is None:
                main_body()
            else:
                with tc.For_i(0, loop_reps, 1):
                    main_body()

    if coalesce:
        _coalesce_sem_incs(nc)
    _legalize_waits(nc)
    return nc


def _coalesce_sem_incs(nc):
    """Coalesce per-instruction semaphore increments.

    Tile attaches a +1 sem-inc to every PE/DVE/ACT instruction for buffer
    rotation tracking; serialized EVT_SEM register writes cost ~26ns each
    (404 matmul incs ~= 10us here). Engines complete instructions in order
    and waits are immediate thresholds, so a semaphore only needs its
    increments at instructions whose cumulative count crosses some waited
    threshold. Walrus requires UpdateValue == 1, so instead of bulk incs we
    keep one +1 at each threshold-crossing instruction and renumber every
    wait threshold to count kept increments: the semaphore reaches each
    (renumbered) threshold at exactly the same instruction completion as
    before. Only applied to semaphores whose increments all come from one
    engine's non-DMA instructions in one block, are all sem-inc(+1)
    immediates, and whose waits are all sem-ge-imm (resets, decs or
    register operands disqualify the semaphore)."""
    waits = {}
    incs = {}
    bad = set()
    for f in nc.m.functions:
        for bi, blk in enumerate(f.blocks):
            for inst in blk.instructions:
                si = getattr(inst, "sync_info", None)
                if si is None:
                    continue
                for w in si.on_wait or []:
                    if w.sync_type != "semaphore":
                        continue
                    if w.wait_mode != "sem-ge-imm" or w.wait_value is None:
                        bad.add(w.id)
                    else:
                        waits.setdefault(w.id, []).append(w)
                is_dma = "DMA" in type(inst).__name__.upper()
                for u in si.on_update or []:
                    if u.sync_type != "semaphore":
                        continue
                    if (
                        u.update_mode != "sem-inc"
                        or u.update_reg is not None
                        or u.update_value != 1
                        or is_dma
                    ):
                        bad.add(u.id)
                    else:
                        incs.setdefault(u.id, []).append((bi, inst.engine, inst, u))
    import bisect

    for sid, lst in incs.items():
        if sid in bad:
            continue
        if len({bi for bi, _, _, _ in lst}) != 1:
            continue
        if len({str(e) for _, e, _, _ in lst}) != 1:
            continue
        if len(lst) < 8:
            continue
        wlist = waits.get(sid, [])
        thresholds = sorted({w.wait_value for w in wlist})
        if thresholds and thresholds[-1] > len(lst):
            continue  # wait beyond total: leave untouched
        kept_cums = []
        ti = 0
        for idx in range(len(lst)):
            cum = idx + 1
            if ti < len(thresholds) and thresholds[ti] <= cum:
                kept_cums.append(cum)
                while ti < len(thresholds) and thresholds[ti] <= cum:
                    ti += 1
        keep_idx = {c - 1 for c in kept_cums}
        for idx, (_, _, inst, u) in enumerate(lst):
            if idx not in keep_idx:
                inst.sync_info.on_update = [
                    x for x in inst.sync_info.on_update if x is not u
                ]
        for w in wlist:
            w.wait_value = bisect.bisect_left(kept_cums, w.wait_value) + 1


def _legalize_waits(nc):
    """Walrus allows only one sem wait on most instruction structs (matmul
    weight-load, DVE/ACT compute, pseudo-DMA, drain). Tile sometimes emits
    more. Fix: hoist excess waits onto standalone EventSemaphore instructions
    inserted just before the owner in its engine stream (same engine ->
    in-order issue preserves semantics)."""
    n = 0
    for f in nc.m.functions:
        for blk in f.blocks:
            out = []
            changed = False
            for i in blk.instructions:
                si = getattr(i, "sync_info", None)
                ow = list(si.on_wait) if (si is not None and si.on_wait) else []
                if len(ow) > 1:
                    while len(ow) > 1:
                        w = ow.pop(0)
                        out.append(
                            mybir.InstEventSemaphore(
                                name=f"I-whoist-{n}",
                                engine=i.engine,
                                ins=[],
                                outs=[],
                                sync_info=mybir.SyncInfo(on_wait=[w], on_update=[]),
                            )
                        )
                        n += 1
                    si.on_wait = ow
                    changed = True
                out.append(i)
            if changed:
                blk.instructions = out
    return nc


def _bf16_split(a_f32):
    """Split fp32 array into (hi, lo) bf16 halves, RNE, as bf16 views."""
    import ml_dtypes

    def rne_bf16(f):
        bits = f.view(np.uint32)
        lsb = (bits >> np.uint32(16)) & np.uint32(1)
        rnd = bits + np.uint32(0x7FFF) + lsb
        return (rnd >> np.uint32(16)).astype(np.uint16)

    hi_u16 = rne_bf16(a_f32)
    hi_f32 = (hi_u16.astype(np.uint32) << np.uint32(16)).view(np.float32)
    lo_u16 = rne_bf16(a_f32 - hi_f32)
    return hi_u16.view(ml_dtypes.bfloat16), lo_u16.view(ml_dtypes.bfloat16)


_NC = None


def _get_nc():
    global _NC
    if _NC is None:
        _NC = build_bass()
    return _NC


def host_prepare(hidden_states, weight):
    """Shard + transpose + bf16-split the full inputs into per-core maps."""
    hs = np.asarray(hidden_states, dtype=np.float32).reshape(T_TOTAL, H)
    wt = np.ascontiguousarray(np.asarray(weight, dtype=np.float32).T)  # [H, E]
    wh_u, wl_u = _bf16_split(wt)
    wh3 = np.ascontiguousarray(wh_u.reshape(N_KC, P, E))
    wl3 = np.ascontiguousarray(wl_u.reshape(N_KC, P, E))
    whl3 = np.ascontiguousarray(np.concatenate([wh3, wl3], axis=2))

    in_maps = []
    for c in range(N_CORES):
        xc = np.ascontiguousarray(hs[c * T_CORE : (c + 1) * T_CORE, :].T)  # [H, Tc]
        xh_u, xl_u = _bf16_split(xc)

        def slab_major(a):
            a4 = a.reshape(N_KC, P, N_SLAB, SLAB).transpose(2, 1, 0, 3)
            return np.ascontiguousarray(a4)

        xh4 = slab_major(xh_u)
        xl4 = slab_major(xl_u)
        in_maps.append(
            {
                "xh": xh4,
                "xl": xl4,
                "x2": np.ascontiguousarray(np.stack([xh4, xl4], axis=2)),
                "wh": wh3,
                "wl": wl3,
                "whl": whl3,
            }
        )
    return in_maps


def kernel(hidden_states, weight, **run_kwargs):
    in_maps = host_prepare(hidden_states, weight)
    nc = _get_nc()
    res = run_bass_kernel_spmd(nc, in_maps, core_ids=list(range(N_CORES)), **run_kwargs)
    topk_weight = np.concatenate([r["out_w"] for r in res.results], axis=0)
    topk_idx = np.concatenate(
        [r["out_i"].astype(np.int32) for r in res.results], axis=0
    )
    if run_kwargs:
        kernel.last_result = res
    return topk_weight, topk_idx
